# revision 4
# baseline (speedup 1.0000x reference)
import sys
import numpy as np

sys.path.insert(0, "/opt/trn_rl_repo")

NEG = 0.2
GH = 10
ELEM0 = 384
ELEM1 = 128



def _prep(src, dst, n_pad, n_cores, gh):
    import ml_dtypes

    per = n_pad // n_cores
    half_r = per // 2
    nmac = per // 128
    cap = gh * 128

    order = np.argsort(dst, kind="stable")
    ds = dst[order].astype(np.int64)
    ss = src[order].astype(np.int64)

    src_core = ss // per
    src_row = ss % per
    is_lo = src_row < half_r
    rowid = np.where(is_lo,
                     half_r * src_core + src_row,
                     half_r * src_core + src_row - half_r).astype(np.int64)
    mac = ds // 128
    drel = (ds - mac * 128).astype(np.int64)

    cores = {}
    for c in range(n_cores):
        out = {}
        for half in (0, 1):
            sel_half = is_lo if half == 0 else ~is_lo
            idxv = np.zeros((nmac, cap), np.int64)
            drl = np.full((nmac, cap), -1.0, np.float32)
            for m in range(nmac):
                gm = c * nmac + m
                sel = (mac == gm) & sel_half
                k = int(sel.sum())
                if k > cap:
                    raise ValueError(f"macro {gm} half {half}: {k} > {cap} edges")
                idxv[m, :k] = rowid[sel]
                drl[m, :k] = drel[sel]
            iv = idxv.reshape(-1)
            dv = drl.reshape(-1)

            nsub = nmac * gh
            ncall = (nsub + 7) // 8
            idx_arr = np.zeros((128, ncall * 64), np.int16)
            dr_arr = np.full((128, ncall * 8), -1.0, np.float32)
            for k in range(ncall):
                t0 = k * 1024
                buf = np.zeros(1024, np.int64)
                nt = min(1024, len(iv) - t0)
                buf[:nt] = iv[t0:t0 + nt]
                idx_arr[:, k * 64:(k + 1) * 64] = np.tile(
                    buf.reshape(64, 16).T.astype(np.int16), (8, 1))
                dbuf = np.full(1024, -1.0, np.float32)
                dbuf[:nt] = dv[t0:t0 + nt]
                dr_arr[:, k * 8:(k + 1) * 8] = dbuf.reshape(8, 128).T
            out[half] = (idx_arr, dr_arr.astype(ml_dtypes.bfloat16))
        cores[c] = out

    return {
        "per": per, "half_r": half_r, "nmac": nmac, "gh": gh,
        "nsub": nmac * gh, "ncall": (nmac * gh + 7) // 8, "cores": cores,
    }



def _build(meta, fdim0, hd0, fdim1, hd1):
    import concourse.bacc as bacc
    import concourse.mybir as mybir
    import concourse.tile as tile
    from concourse.masks import make_identity

    NC = 8
    per, half_r = meta["per"], meta["half_r"]
    nmac, gh = meta["nmac"], meta["gh"]
    ncall, nsub = meta["ncall"], meta["nsub"]

    H0, D0 = hd0
    H1, D1 = hd1
    C0 = fdim0 + H0
    C1 = fdim1 + H1
    W0N = fdim0 + 2 * H0
    W1N = fdim1 + 2 * H1 + fdim1
    KD = fdim0
    NK = KD // 128

    f32 = mybir.dt.float32
    bf16 = mybir.dt.bfloat16
    i16 = mybir.dt.int16

    nc = bacc.Bacc("TRN2", target_bir_lowering=False, debug=False,
                   num_devices=NC, num_swdge_queues=4)

    xts = nc.declare_dram_parameter("xts", [KD, per], f32, isOutput=False)
    w0e = nc.declare_dram_parameter("w0e", [KD, W0N], f32, isOutput=False)
    w1e = nc.declare_dram_parameter("w1e", [fdim0, W1N], f32, isOutput=False)
    idxlo = nc.declare_dram_parameter("idxlo", [128, ncall * 64], i16, isOutput=False)
    idxhi = nc.declare_dram_parameter("idxhi", [128, ncall * 64], i16, isOutput=False)
    drlo = nc.declare_dram_parameter("drlo", [128, ncall * 8], bf16, isOutput=False)
    drhi = nc.declare_dram_parameter("drhi", [128, ncall * 8], bf16, isOutput=False)
    out_d = nc.declare_dram_parameter("out", [per, fdim1], f32, isOutput=True)

    t0_own = nc.dram_tensor("t0_own", [per, ELEM0], bf16)
    t0_lo = nc.dram_tensor("t0_lo", [NC * half_r, ELEM0], bf16, addr_space="Shared")
    t0_hi = nc.dram_tensor("t0_hi", [NC * half_r, ELEM0], bf16, addr_space="Shared")
    t1_own = nc.dram_tensor("t1_own", [per, ELEM1], bf16)
    t1_lo = nc.dram_tensor("t1_lo", [NC * half_r, ELEM1], bf16, addr_space="Shared")
    t1_hi = nc.dram_tensor("t1_hi", [NC * half_r, ELEM1], bf16, addr_space="Shared")
    er0_own = nc.dram_tensor("er0_own", [per, H0], f32)
    er1_own = nc.dram_tensor("er1_own", [per, H1], f32)

    rg = [list(range(NC))]

    def call_slots(k):
        out = []
        for s in range(8):
            p = 8 * k + s
            if p >= nsub:
                break
            out.append((s, p // gh, p % gh))
        return out

    with tile.TileContext(nc) as tc:
        with tc.tile_pool(name="consts", bufs=1) as consts:
            ident_bf = consts.tile([128, 128], bf16)
            make_identity(nc, ident_bf[:])
            iota_bf = consts.tile([128, 128], bf16)
            nc.gpsimd.iota(iota_bf[:], pattern=[[1, 128]], base=0,
                           channel_multiplier=0,
                           allow_small_or_imprecise_dtypes=True)
            w0s = consts.tile([128, NK * W0N], f32)
            for j in range(NK):
                nc.sync.dma_start(w0s[:, j * W0N:(j + 1) * W0N],
                                  w0e[128 * j:128 * (j + 1), :])
            w1s = consts.tile([128, NK * W1N], f32)
            for j in range(NK):
                nc.sync.dma_start(w1s[:, j * W1N:(j + 1) * W1N],
                                  w1e[128 * j:128 * (j + 1), :])
            w1sb = consts.tile([128, NK * W1N], bf16)
            nc.vector.tensor_copy(w1sb[:], w1s[:])
            idxlo_t = consts.tile([128, ncall * 64], i16)
            nc.sync.dma_start(idxlo_t[:], idxlo[:])
            idxhi_t = consts.tile([128, ncall * 64], i16)
            nc.sync.dma_start(idxhi_t[:], idxhi[:])
            drlo_t = consts.tile([128, ncall * 8], bf16)
            nc.sync.dma_start(drlo_t[:], drlo[:])
            drhi_t = consts.tile([128, ncall * 8], bf16)
            nc.sync.dma_start(drhi_t[:], drhi[:])
            res_sb = consts.tile([128, nmac * fdim1], f32)

            with (
                tc.tile_pool(name="pa", bufs=3) as pa,
                tc.tile_pool(name="paps", bufs=2, space="PSUM") as paps,
            ):
                for t in range(nmac):
                    ps = paps.tile([128, W0N], f32, tag="psA")
                    for j in range(NK):
                        xt = pa.tile([128, 128], f32, tag="xt")
                        nc.sync.dma_start(
                            xt[:],
                            xts[128 * j:128 * (j + 1), 128 * t:128 * (t + 1)])
                        nc.tensor.matmul(ps[:], lhsT=xt[:],
                                         rhs=w0s[:, j * W0N:(j + 1) * W0N],
                                         start=(j == 0), stop=(j == NK - 1))
                    sb = pa.tile([128, ELEM0], bf16, tag="sbA")
                    nc.vector.memset(sb[:, C0:ELEM0], 0.0)
                    nc.vector.tensor_copy(sb[:, 0:C0], ps[:, 0:C0])
                    nc.sync.dma_start(t0_own[128 * t:128 * (t + 1), :], sb[:])
                    se = pa.tile([128, H0], f32, tag="seA")
                    nc.vector.tensor_copy(se[:], ps[:, fdim0 + H0:fdim0 + 2 * H0])
                    nc.sync.dma_start(er0_own[128 * t:128 * (t + 1), :], se[:])

            nc.gpsimd.collective_compute(
                "AllGather", mybir.AluOpType.bypass, replica_groups=rg,
                ins=[t0_own[0:half_r, :]], outs=[t0_lo[:]])
            nc.gpsimd.collective_compute(
                "AllGather", mybir.AluOpType.bypass, replica_groups=rg,
                ins=[t0_own[half_r:per, :]], outs=[t0_hi[:]])

            qctr = [0]

            def edge_loop(tlo, thi, elem, fdim, H, er_own, epilogue):
                D = fdim // H
                C = fdim + H
                with (
                    tc.tile_pool(name="gp", bufs=6) as gp,
                    tc.tile_pool(name="wp", bufs=3) as wp,
                    tc.tile_pool(name="ep", bufs=3) as ep,
                    tc.tile_pool(name="pps", bufs=2, space="PSUM") as pps,
                    tc.tile_pool(name="opsp", bufs=2, space="PSUM") as opsp,
                ):
                    er_bands = {}
                    out_ps = {}

                    def ensure_macro(m):
                        if m in out_ps:
                            return
                        out_ps[m] = opsp.tile([128, C], f32, tag="outps", name=f"ops{m}")
                        eb = ep.tile([128, H], f32, tag="erb")
                        nc.sync.dma_start(eb[:], er_own[128 * m:128 * (m + 1), :])
                        ebb = ep.tile([128, H], bf16, tag="erbb")
                        nc.vector.tensor_copy(ebb[:], eb[:])
                        er_bands[m] = ebb

                    def emit_call(half, k, idx_t, dr_t, tbl):
                        slots = call_slots(k)
                        if not slots:
                            return
                        ns = len(slots)
                        nt = 128 * ns
                        for (_, m, _) in slots:
                            ensure_macro(m)
                        g = gp.tile([128, 8 * elem], bf16, tag="g")
                        nc.gpsimd.dma_gather(
                            idxs_ap=idx_t[:, 64 * k:64 * k + (nt // 16)],
                            in_ap=tbl[:],
                            out_ap=g[:, 0:ns * elem].rearrange(
                                "p (s e) -> p s e", e=elem),
                            num_idxs=nt, num_idxs_reg=nt, elem_size=elem,
                            queue_num=qctr[0] % 4,
                        )
                        qctr[0] += 1
                        gv = g[:].rearrange("p (s e) -> p s e", e=elem)

                        B = wp.tile([128, 8 * 128], bf16, tag="B")
                        nc.vector.tensor_tensor(
                            out=B[:, 0:ns * 128].rearrange("p (s n) -> p s n", n=128),
                            in0=iota_bf[:].unsqueeze(1).broadcast_to([128, ns, 128]),
                            in1=dr_t[:, 8 * k:8 * k + ns].unsqueeze(2)
                                .broadcast_to([128, ns, 128]),
                            op=mybir.AluOpType.is_equal)

                        BT = wp.tile([128, 8 * 128], bf16, tag="BT")
                        for base in range(0, ns, 4):
                            top = min(base + 4, ns)
                            bt_ps = pps.tile([128, 512], bf16, tag="btps")
                            for s in range(base, top):
                                nc.tensor.transpose(
                                    out=bt_ps[:, (s - base) * 128:(s - base + 1) * 128],
                                    in_=B[:, s * 128:(s + 1) * 128],
                                    identity=ident_bf[:])
                            nc.vector.tensor_copy(
                                BT[:, base * 128:top * 128],
                                bt_ps[:, 0:(top - base) * 128])

                        erp = pps.tile([128, 8 * H], f32, tag="small")
                        for (s, m, _) in slots:
                            nc.tensor.matmul(
                                erp[:, s * H:(s + 1) * H],
                                lhsT=BT[:, s * 128:(s + 1) * 128],
                                rhs=er_bands[m][:], start=True, stop=True)

                        e_t = ep.tile([128, 8 * H], f32, tag="e")
                        nc.vector.tensor_tensor(
                            out=e_t[:, 0:ns * H].rearrange("p (s h) -> p s h", h=H),
                            in0=gv[:, 0:ns, fdim:fdim + H],
                            in1=erp[:, 0:ns * H].rearrange("p (s h) -> p s h", h=H),
                            op=mybir.AluOpType.add)
                        ab = ep.tile([128, 8 * H], f32, tag="abs")
                        nc.scalar.activation(ab[:, 0:ns * H], e_t[:, 0:ns * H],
                                             mybir.ActivationFunctionType.Abs,
                                             scale=(1.0 - NEG) / 2.0)
                        lr = ep.tile([128, 8 * H], f32, tag="lr")
                        nc.vector.scalar_tensor_tensor(
                            out=lr[:, 0:ns * H], in0=e_t[:, 0:ns * H],
                            scalar=(1.0 + NEG) / 2.0, in1=ab[:, 0:ns * H],
                            op0=mybir.AluOpType.mult, op1=mybir.AluOpType.add)
                        exb = ep.tile([128, 8 * H], bf16, tag="exb")
                        nc.scalar.activation(exb[:, 0:ns * H], lr[:, 0:ns * H],
                                             mybir.ActivationFunctionType.Exp)

                        rhs = wp.tile([128, 8 * C], bf16, tag="rhs")
                        rv = rhs[:].rearrange("p (s c) -> p s c", c=C)
                        nc.vector.tensor_tensor(
                            out=rv[:, 0:ns, 0:fdim].rearrange(
                                "p s (h d) -> p s h d", d=D),
                            in0=gv[:, 0:ns, 0:fdim].rearrange(
                                "p s (h d) -> p s h d", d=D),
                            in1=exb[:, 0:ns * H].rearrange("p (s h) -> p s h", h=H)
                                .unsqueeze(3).broadcast_to([128, ns, H, D]),
                            op=mybir.AluOpType.mult)
                        nc.vector.tensor_copy(
                            rv[:, 0:ns, fdim:C],
                            exb[:, 0:ns * H].rearrange("p (s h) -> p s h", h=H))

                        for (s, m, j) in slots:
                            nc.tensor.matmul(
                                out_ps[m][:], lhsT=B[:, s * 128:(s + 1) * 128],
                                rhs=rhs[:, s * C:(s + 1) * C],
                                start=(half == 0 and j == 0),
                                stop=(half == 1 and j == gh - 1))

                    for k in range(ncall):
                        emit_call(0, k, idxlo_t, drlo_t, tlo)
                        emit_call(1, k, idxhi_t, drhi_t, thi)
                        for (_, m, j) in call_slots(k):
                            if j == gh - 1:
                                epilogue(m, out_ps.pop(m), pps, ep)

            def epi0(m, ops, pps, ep):
                dmx = ep.tile([128, H0], f32, tag="dmx")
                nc.vector.tensor_scalar_max(dmx[:], ops[:, fdim0:fdim0 + H0], 1e-30)
                r = ep.tile([128, H0], f32, tag="r")
                nc.vector.reciprocal(r[:], dmx[:])
                hn = ep.tile([128, fdim0], f32, tag="hn")
                nc.vector.tensor_tensor(
                    out=hn[:].rearrange("p (h d) -> p h d", d=D0),
                    in0=ops[:, 0:fdim0].rearrange("p (h d) -> p h d", d=D0),
                    in1=r[:].unsqueeze(2).broadcast_to([128, H0, D0]),
                    op=mybir.AluOpType.mult)
                mt = ep.tile([128, fdim0], f32, tag="mt")
                nc.vector.tensor_scalar_min(mt[:], hn[:], 0.0)
                em = ep.tile([128, fdim0], f32, tag="em")
                nc.scalar.activation(em[:], mt[:], mybir.ActivationFunctionType.Exp)
                pt = ep.tile([128, fdim0], f32, tag="pt")
                nc.vector.tensor_scalar_max(pt[:], hn[:], 0.0)
                h = ep.tile([128, fdim0], f32, tag="h")
                nc.vector.scalar_tensor_tensor(
                    out=h[:], in0=em[:], scalar=-1.0, in1=pt[:],
                    op0=mybir.AluOpType.add, op1=mybir.AluOpType.add)

                h_bf = ep.tile([128, fdim0], bf16, tag="h_bf")
                nc.vector.tensor_copy(h_bf[:], h[:])
                ht_ps = pps.tile([128, 512], bf16, tag="btps")
                for j in range(NK):
                    nc.tensor.transpose(out=ht_ps[:, j * 128:(j + 1) * 128],
                                        in_=h_bf[:, j * 128:(j + 1) * 128],
                                        identity=ident_bf[:])
                ht = ep.tile([128, fdim0], bf16, tag="ht")
                nc.vector.tensor_copy(ht[:], ht_ps[:, 0:fdim0])
                ps1 = pps.tile([128, W1N], f32, tag="small2")
                for j in range(NK):
                    nc.tensor.matmul(ps1[:], lhsT=ht[:, j * 128:(j + 1) * 128],
                                     rhs=w1sb[:, j * W1N:(j + 1) * W1N],
                                     start=(j == 0), stop=(j == NK - 1))
                t1row = ep.tile([128, ELEM1], bf16, tag="t1row")
                nc.vector.memset(t1row[:, C1:ELEM1], 0.0)
                nc.vector.tensor_copy(t1row[:, 0:C1], ps1[:, 0:C1])
                nc.sync.dma_start(t1_own[128 * m:128 * (m + 1), :], t1row[:])
                se1 = ep.tile([128, H1], f32, tag="se1")
                nc.vector.tensor_copy(se1[:], ps1[:, fdim1 + H1:fdim1 + 2 * H1])
                nc.sync.dma_start(er1_own[128 * m:128 * (m + 1), :], se1[:])
                nc.vector.tensor_copy(
                    res_sb[:, fdim1 * m:fdim1 * (m + 1)],
                    ps1[:, fdim1 + 2 * H1:W1N])

            edge_loop(t0_lo, t0_hi, ELEM0, fdim0, H0, er0_own, epi0)

            nc.gpsimd.collective_compute(
                "AllGather", mybir.AluOpType.bypass, replica_groups=rg,
                ins=[t1_own[0:half_r, :]], outs=[t1_lo[:]])
            nc.gpsimd.collective_compute(
                "AllGather", mybir.AluOpType.bypass, replica_groups=rg,
                ins=[t1_own[half_r:per, :]], outs=[t1_hi[:]])

            def epi1(m, ops, pps, ep):
                dmx = ep.tile([128, H1], f32, tag="dmx1")
                nc.vector.tensor_scalar_max(dmx[:], ops[:, fdim1:fdim1 + H1], 1e-30)
                r = ep.tile([128, H1], f32, tag="r1")
                nc.vector.reciprocal(r[:], dmx[:])
                ot = ep.tile([128, fdim1], f32, tag="ot")
                nc.vector.scalar_tensor_tensor(
                    out=ot[:], in0=ops[:, 0:fdim1], scalar=r[:, 0:1],
                    in1=res_sb[:, fdim1 * m:fdim1 * (m + 1)],
                    op0=mybir.AluOpType.mult, op1=mybir.AluOpType.add)
                nc.sync.dma_start(out_d[128 * m:128 * (m + 1), :], ot[:])

            edge_loop(t1_lo, t1_hi, ELEM1, fdim1, H1, er1_own, epi1)

    nc.compile()
    return nc



def kernel(x, src, dst, W0, al0, ar0, W1, al1, ar1, Wres):
    from concourse.bass_utils import run_bass_kernel_spmd

    x = np.asarray(x, np.float32)
    src = np.asarray(src, np.int32)
    dst = np.asarray(dst, np.int32)
    W0 = np.asarray(W0, np.float32)
    al0 = np.asarray(al0, np.float32)
    ar0 = np.asarray(ar0, np.float32)
    W1 = np.asarray(W1, np.float32)
    al1 = np.asarray(al1, np.float32)
    ar1 = np.asarray(ar1, np.float32)
    Wres = np.asarray(Wres, np.float32)

    N, F = x.shape
    NC = 8
    n_pad = ((N + NC * 128 - 1) // (NC * 128)) * (NC * 128)
    H0, D0 = al0.shape
    H1, D1 = al1.shape
    fdim0, fdim1 = H0 * D0, H1 * D1

    meta = _prep(src, dst, n_pad, NC, gh=GH)
    per = meta["per"]

    w0e = np.concatenate([
        W0,
        (W0.reshape(F, H0, D0) * al0[None]).sum(-1),
        (W0.reshape(F, H0, D0) * ar0[None]).sum(-1),
    ], axis=1).astype(np.float32)
    w1e = np.concatenate([
        W1,
        (W1.reshape(fdim0, H1, D1) * al1[None]).sum(-1),
        (W1.reshape(fdim0, H1, D1) * ar1[None]).sum(-1),
        Wres,
    ], axis=1).astype(np.float32)

    x_pad = np.zeros((n_pad, F), np.float32)
    x_pad[:N] = x
    xT = np.ascontiguousarray(x_pad.T)

    nc = _build(meta, fdim0, (H0, D0), fdim1, (H1, D1))

    in_maps = []
    for c in range(NC):
        ilo, dlo = meta["cores"][c][0]
        ihi, dhi = meta["cores"][c][1]
        in_maps.append({
            "xts": np.ascontiguousarray(xT[:, c * per:(c + 1) * per]),
            "w0e": w0e, "w1e": w1e,
            "idxlo": ilo, "idxhi": ihi,
            "drlo": dlo, "drhi": dhi,
        })

    res = run_bass_kernel_spmd(nc, in_maps, core_ids=list(range(NC)))
    out = np.concatenate([res.results[c]["out"] for c in range(NC)], axis=0)
    kernel.last_exec_ns = res.exec_time_ns
    return out[:N].astype(np.float32)


kernel.last_exec_ns = None


# revision 7
# speedup vs baseline: 1.1316x; 1.1316x over previous
import sys
import numpy as np

sys.path.insert(0, "/opt/trn_rl_repo")

NEG = 0.2
GH = 10
ELEM0 = 384
ELEM1 = 128



def _prep(src, dst, n_pad, n_cores, gh):
    import ml_dtypes

    per = n_pad // n_cores
    half_r = per // 2
    nmac = per // 128
    cap = gh * 128

    order = np.argsort(dst, kind="stable")
    ds = dst[order].astype(np.int64)
    ss = src[order].astype(np.int64)

    src_core = ss // per
    src_row = ss % per
    is_lo = src_row < half_r
    rowid = np.where(is_lo,
                     half_r * src_core + src_row,
                     half_r * src_core + src_row - half_r).astype(np.int64)
    mac = ds // 128
    drel = (ds - mac * 128).astype(np.int64)

    cores = {}
    for c in range(n_cores):
        out = {}
        for half in (0, 1):
            sel_half = is_lo if half == 0 else ~is_lo
            idxv = np.zeros((nmac, cap), np.int64)
            drl = np.full((nmac, cap), -1.0, np.float32)
            for m in range(nmac):
                gm = c * nmac + m
                sel = (mac == gm) & sel_half
                k = int(sel.sum())
                if k > cap:
                    raise ValueError(f"macro {gm} half {half}: {k} > {cap} edges")
                idxv[m, :k] = rowid[sel]
                drl[m, :k] = drel[sel]
            iv = idxv.reshape(-1)
            dv = drl.reshape(-1)

            nsub = nmac * gh
            ncall = (nsub + 7) // 8
            idx_arr = np.zeros((128, ncall * 64), np.int16)
            dr_arr = np.full((128, ncall * 8), -1.0, np.float32)
            drt_arr = np.full((128, ncall * 1024), -1.0, np.float32)
            for k in range(ncall):
                t0 = k * 1024
                buf = np.zeros(1024, np.int64)
                nt = min(1024, len(iv) - t0)
                buf[:nt] = iv[t0:t0 + nt]
                idx_arr[:, k * 64:(k + 1) * 64] = np.tile(
                    buf.reshape(64, 16).T.astype(np.int16), (8, 1))
                dbuf = np.full(1024, -1.0, np.float32)
                dbuf[:nt] = dv[t0:t0 + nt]
                dr_arr[:, k * 8:(k + 1) * 8] = dbuf.reshape(8, 128).T
                drt_arr[:, k * 1024:(k + 1) * 1024] = dbuf[None, :]
            out[half] = (idx_arr, dr_arr.astype(ml_dtypes.bfloat16),
                         drt_arr.astype(ml_dtypes.bfloat16))
        cores[c] = out

    return {
        "per": per, "half_r": half_r, "nmac": nmac, "gh": gh,
        "nsub": nmac * gh, "ncall": (nmac * gh + 7) // 8, "cores": cores,
    }



def _build(meta, fdim0, hd0, fdim1, hd1):
    import concourse.bacc as bacc
    import concourse.mybir as mybir
    import concourse.tile as tile
    from concourse.masks import make_identity

    NC = 8
    per, half_r = meta["per"], meta["half_r"]
    nmac, gh = meta["nmac"], meta["gh"]
    ncall, nsub = meta["ncall"], meta["nsub"]

    H0, D0 = hd0
    H1, D1 = hd1
    C0 = fdim0 + H0
    C1 = fdim1 + H1
    W0N = fdim0 + 2 * H0
    W1N = fdim1 + 2 * H1 + fdim1
    KD = fdim0
    NK = KD // 128

    f32 = mybir.dt.float32
    bf16 = mybir.dt.bfloat16
    i16 = mybir.dt.int16

    nc = bacc.Bacc("TRN2", target_bir_lowering=False, debug=False,
                   num_devices=NC, num_swdge_queues=4,
                   dynamic_dma_scratch_size=65536)

    xts = nc.declare_dram_parameter("xts", [KD, per], f32, isOutput=False)
    w0e = nc.declare_dram_parameter("w0e", [KD, W0N], f32, isOutput=False)
    w1e = nc.declare_dram_parameter("w1e", [fdim0, W1N], f32, isOutput=False)
    idxlo = nc.declare_dram_parameter("idxlo", [128, ncall * 64], i16, isOutput=False)
    idxhi = nc.declare_dram_parameter("idxhi", [128, ncall * 64], i16, isOutput=False)
    drlo = nc.declare_dram_parameter("drlo", [128, ncall * 8], bf16, isOutput=False)
    drhi = nc.declare_dram_parameter("drhi", [128, ncall * 8], bf16, isOutput=False)
    drtlo = nc.declare_dram_parameter("drtlo", [128, ncall * 1024], bf16, isOutput=False)
    drthi = nc.declare_dram_parameter("drthi", [128, ncall * 1024], bf16, isOutput=False)
    out_d = nc.declare_dram_parameter("out", [per, fdim1], f32, isOutput=True)

    t0_own = nc.dram_tensor("t0_own", [per, ELEM0], bf16)
    t0_lo = nc.dram_tensor("t0_lo", [NC * half_r, ELEM0], bf16, addr_space="Shared")
    t0_hi = nc.dram_tensor("t0_hi", [NC * half_r, ELEM0], bf16, addr_space="Shared")
    t1_own = nc.dram_tensor("t1_own", [per, ELEM1], bf16)
    t1_lo = nc.dram_tensor("t1_lo", [NC * half_r, ELEM1], bf16, addr_space="Shared")
    t1_hi = nc.dram_tensor("t1_hi", [NC * half_r, ELEM1], bf16, addr_space="Shared")
    er0_own = nc.dram_tensor("er0_own", [per, H0], f32)
    er1_own = nc.dram_tensor("er1_own", [per, H1], f32)

    rg = [list(range(NC))]

    def call_slots(k):
        out = []
        for s in range(8):
            p = 8 * k + s
            if p >= nsub:
                break
            out.append((s, p // gh, p % gh))
        return out

    with tile.TileContext(nc) as tc:
        with tc.tile_pool(name="consts", bufs=1) as consts:
            ident_bf = consts.tile([128, 128], bf16)
            make_identity(nc, ident_bf[:])
            iota_bf = consts.tile([128, 128], bf16)
            nc.gpsimd.iota(iota_bf[:], pattern=[[1, 128]], base=0,
                           channel_multiplier=0,
                           allow_small_or_imprecise_dtypes=True)
            iota_col = consts.tile([128, 1], f32)
            nc.gpsimd.iota(iota_col[:], pattern=[[1, 1]], base=0,
                           channel_multiplier=1,
                           allow_small_or_imprecise_dtypes=True)
            w0s = consts.tile([128, NK * W0N], f32)
            for j in range(NK):
                nc.sync.dma_start(w0s[:, j * W0N:(j + 1) * W0N],
                                  w0e[128 * j:128 * (j + 1), :])
            w1s = consts.tile([128, NK * W1N], f32)
            for j in range(NK):
                nc.sync.dma_start(w1s[:, j * W1N:(j + 1) * W1N],
                                  w1e[128 * j:128 * (j + 1), :])
            w1sb = consts.tile([128, NK * W1N], bf16)
            nc.vector.tensor_copy(w1sb[:], w1s[:])
            idxlo_t = consts.tile([128, ncall * 64], i16)
            nc.sync.dma_start(idxlo_t[:], idxlo[:])
            idxhi_t = consts.tile([128, ncall * 64], i16)
            nc.sync.dma_start(idxhi_t[:], idxhi[:])
            drlo_t = consts.tile([128, ncall * 8], bf16)
            nc.sync.dma_start(drlo_t[:], drlo[:])
            drhi_t = consts.tile([128, ncall * 8], bf16)
            nc.sync.dma_start(drhi_t[:], drhi[:])
            res_sb = consts.tile([128, nmac * fdim1], f32)

            with (
                tc.tile_pool(name="pa", bufs=3) as pa,
                tc.tile_pool(name="paps", bufs=2, space="PSUM") as paps,
            ):
                for t in range(nmac):
                    ps = paps.tile([128, W0N], f32, tag="psA")
                    for j in range(NK):
                        xt = pa.tile([128, 128], f32, tag="xt")
                        nc.sync.dma_start(
                            xt[:],
                            xts[128 * j:128 * (j + 1), 128 * t:128 * (t + 1)])
                        nc.tensor.matmul(ps[:], lhsT=xt[:],
                                         rhs=w0s[:, j * W0N:(j + 1) * W0N],
                                         start=(j == 0), stop=(j == NK - 1))
                    sb = pa.tile([128, ELEM0], bf16, tag="sbA")
                    nc.vector.memset(sb[:, C0:ELEM0], 0.0)
                    nc.vector.tensor_copy(sb[:, 0:C0], ps[:, 0:C0])
                    nc.sync.dma_start(t0_own[128 * t:128 * (t + 1), :], sb[:])
                    se = pa.tile([128, H0], f32, tag="seA")
                    nc.vector.tensor_copy(se[:], ps[:, fdim0 + H0:fdim0 + 2 * H0])
                    nc.sync.dma_start(er0_own[128 * t:128 * (t + 1), :], se[:])

            nc.gpsimd.collective_compute(
                "AllGather", mybir.AluOpType.bypass, replica_groups=rg,
                ins=[t0_own[0:half_r, :]], outs=[t0_lo[:]])
            nc.gpsimd.collective_compute(
                "AllGather", mybir.AluOpType.bypass, replica_groups=rg,
                ins=[t0_own[half_r:per, :]], outs=[t0_hi[:]])

            qctr = [0]

            def edge_loop(tlo, thi, elem, fdim, H, er_own, epilogue):
                D = fdim // H
                C = fdim + H
                with (
                    tc.tile_pool(name="gp", bufs=6) as gp,
                    tc.tile_pool(name="wp", bufs=3) as wp,
                    tc.tile_pool(name="ep", bufs=3) as ep,
                    tc.tile_pool(name="pps", bufs=2, space="PSUM") as pps,
                    tc.tile_pool(name="opsp", bufs=2, space="PSUM") as opsp,
                ):
                    er_bands = {}
                    out_ps = {}

                    def ensure_macro(m):
                        if m in out_ps:
                            return
                        out_ps[m] = opsp.tile([128, C], f32, tag="outps", name=f"ops{m}")
                        eb = ep.tile([128, H], f32, tag="erb")
                        nc.sync.dma_start(eb[:], er_own[128 * m:128 * (m + 1), :])
                        ebb = ep.tile([128, H], bf16, tag="erbb")
                        nc.vector.tensor_copy(ebb[:], eb[:])
                        er_bands[m] = ebb

                    def emit_call(half, k, idx_t, dr_t, drt_d, tbl):
                        slots = call_slots(k)
                        if not slots:
                            return
                        ns = len(slots)
                        nt = 128 * ns
                        for (_, m, _) in slots:
                            ensure_macro(m)
                        g = gp.tile([128, 8 * elem], bf16, tag="g")
                        nc.gpsimd.dma_gather(
                            idxs_ap=idx_t[:, 64 * k:64 * k + (nt // 16)],
                            in_ap=tbl[:],
                            out_ap=g[:, 0:ns * elem].rearrange(
                                "p (s e) -> p s e", e=elem),
                            num_idxs=nt, num_idxs_reg=nt, elem_size=elem,
                            queue_num=qctr[0] % 4,
                        )
                        qctr[0] += 1
                        gv = g[:].rearrange("p (s e) -> p s e", e=elem)

                        B = wp.tile([128, 8 * 128], bf16, tag="B")
                        nc.vector.tensor_tensor(
                            out=B[:, 0:ns * 128].rearrange("p (s n) -> p s n", n=128),
                            in0=iota_bf[:].unsqueeze(1).broadcast_to([128, ns, 128]),
                            in1=dr_t[:, 8 * k:8 * k + ns].unsqueeze(2)
                                .broadcast_to([128, ns, 128]),
                            op=mybir.AluOpType.is_equal)

                        drt = wp.tile([128, 8 * 128], bf16, tag="drt")
                        nc.sync.dma_start(drt[:, 0:ns * 128],
                                          drt_d[:, 1024 * k:1024 * k + ns * 128])
                        BT = wp.tile([128, 8 * 128], bf16, tag="BT")
                        nc.vector.tensor_scalar(
                            out=BT[:, 0:ns * 128], in0=drt[:, 0:ns * 128],
                            scalar1=iota_col[:, 0:1], scalar2=None,
                            op0=mybir.AluOpType.is_equal)

                        erp = pps.tile([128, 8 * H], f32, tag="small")
                        for (s, m, _) in slots:
                            nc.tensor.matmul(
                                erp[:, s * H:(s + 1) * H],
                                lhsT=BT[:, s * 128:(s + 1) * 128],
                                rhs=er_bands[m][:], start=True, stop=True)

                        e_t = ep.tile([128, 8 * H], f32, tag="e")
                        nc.vector.tensor_tensor(
                            out=e_t[:, 0:ns * H].rearrange("p (s h) -> p s h", h=H),
                            in0=gv[:, 0:ns, fdim:fdim + H],
                            in1=erp[:, 0:ns * H].rearrange("p (s h) -> p s h", h=H),
                            op=mybir.AluOpType.add)
                        ab = ep.tile([128, 8 * H], f32, tag="abs")
                        nc.scalar.activation(ab[:, 0:ns * H], e_t[:, 0:ns * H],
                                             mybir.ActivationFunctionType.Abs,
                                             scale=(1.0 - NEG) / 2.0)
                        lr = ep.tile([128, 8 * H], f32, tag="lr")
                        nc.vector.scalar_tensor_tensor(
                            out=lr[:, 0:ns * H], in0=e_t[:, 0:ns * H],
                            scalar=(1.0 + NEG) / 2.0, in1=ab[:, 0:ns * H],
                            op0=mybir.AluOpType.mult, op1=mybir.AluOpType.add)
                        rhs = wp.tile([128, 8 * C], bf16, tag="rhs")
                        rv = rhs[:].rearrange("p (s c) -> p s c", c=C)
                        nc.scalar.activation(rv[:, 0:ns, fdim:C],
                                             lr[:, 0:ns * H].rearrange(
                                                 "p (s h) -> p s h", h=H),
                                             mybir.ActivationFunctionType.Exp)
                        nc.vector.tensor_tensor(
                            out=rv[:, 0:ns, 0:fdim].rearrange(
                                "p s (h d) -> p s h d", d=D),
                            in0=gv[:, 0:ns, 0:fdim].rearrange(
                                "p s (h d) -> p s h d", d=D),
                            in1=rv[:, 0:ns, fdim:C]
                                .unsqueeze(3).broadcast_to([128, ns, H, D]),
                            op=mybir.AluOpType.mult)

                        for (s, m, j) in slots:
                            nc.tensor.matmul(
                                out_ps[m][:], lhsT=B[:, s * 128:(s + 1) * 128],
                                rhs=rhs[:, s * C:(s + 1) * C],
                                start=(half == 0 and j == 0),
                                stop=(half == 1 and j == gh - 1))

                    for k in range(ncall):
                        emit_call(0, k, idxlo_t, drlo_t, drtlo, tlo)
                        emit_call(1, k, idxhi_t, drhi_t, drthi, thi)
                        for (_, m, j) in call_slots(k):
                            if j == gh - 1:
                                epilogue(m, out_ps.pop(m), pps, ep)

            def epi0(m, ops, pps, ep):
                dmx = ep.tile([128, H0], f32, tag="dmx")
                nc.vector.tensor_scalar_max(dmx[:], ops[:, fdim0:fdim0 + H0], 1e-30)
                r = ep.tile([128, H0], f32, tag="r")
                nc.vector.reciprocal(r[:], dmx[:])
                hn = ep.tile([128, fdim0], f32, tag="hn")
                nc.vector.tensor_tensor(
                    out=hn[:].rearrange("p (h d) -> p h d", d=D0),
                    in0=ops[:, 0:fdim0].rearrange("p (h d) -> p h d", d=D0),
                    in1=r[:].unsqueeze(2).broadcast_to([128, H0, D0]),
                    op=mybir.AluOpType.mult)
                mt = ep.tile([128, fdim0], f32, tag="mt")
                nc.scalar.activation(mt[:], hn[:],
                                     mybir.ActivationFunctionType.Relu,
                                     scale=-1.0)
                em = ep.tile([128, fdim0], f32, tag="em")
                nc.scalar.activation(em[:], mt[:],
                                     mybir.ActivationFunctionType.Exp,
                                     scale=-1.0)
                pt = ep.tile([128, fdim0], f32, tag="pt")
                nc.scalar.activation(pt[:], hn[:],
                                     mybir.ActivationFunctionType.Relu)
                h = ep.tile([128, fdim0], f32, tag="h")
                nc.vector.scalar_tensor_tensor(
                    out=h[:], in0=em[:], scalar=-1.0, in1=pt[:],
                    op0=mybir.AluOpType.add, op1=mybir.AluOpType.add)

                h_bf = ep.tile([128, fdim0], bf16, tag="h_bf")
                nc.vector.tensor_copy(h_bf[:], h[:])
                ht_ps = pps.tile([128, 512], bf16, tag="btps")
                for j in range(NK):
                    nc.tensor.transpose(out=ht_ps[:, j * 128:(j + 1) * 128],
                                        in_=h_bf[:, j * 128:(j + 1) * 128],
                                        identity=ident_bf[:])
                ht = ep.tile([128, fdim0], bf16, tag="ht")
                nc.vector.tensor_copy(ht[:], ht_ps[:, 0:fdim0])
                ps1 = pps.tile([128, W1N], f32, tag="small2")
                for j in range(NK):
                    nc.tensor.matmul(ps1[:], lhsT=ht[:, j * 128:(j + 1) * 128],
                                     rhs=w1sb[:, j * W1N:(j + 1) * W1N],
                                     start=(j == 0), stop=(j == NK - 1))
                t1row = ep.tile([128, ELEM1], bf16, tag="t1row")
                nc.vector.memset(t1row[:, C1:ELEM1], 0.0)
                nc.vector.tensor_copy(t1row[:, 0:C1], ps1[:, 0:C1])
                nc.sync.dma_start(t1_own[128 * m:128 * (m + 1), :], t1row[:])
                se1 = ep.tile([128, H1], f32, tag="se1")
                nc.vector.tensor_copy(se1[:], ps1[:, fdim1 + H1:fdim1 + 2 * H1])
                nc.sync.dma_start(er1_own[128 * m:128 * (m + 1), :], se1[:])
                nc.vector.tensor_copy(
                    res_sb[:, fdim1 * m:fdim1 * (m + 1)],
                    ps1[:, fdim1 + 2 * H1:W1N])

            edge_loop(t0_lo, t0_hi, ELEM0, fdim0, H0, er0_own, epi0)

            nc.gpsimd.collective_compute(
                "AllGather", mybir.AluOpType.bypass, replica_groups=rg,
                ins=[t1_own[0:half_r, :]], outs=[t1_lo[:]])
            nc.gpsimd.collective_compute(
                "AllGather", mybir.AluOpType.bypass, replica_groups=rg,
                ins=[t1_own[half_r:per, :]], outs=[t1_hi[:]])

            def epi1(m, ops, pps, ep):
                dmx = ep.tile([128, H1], f32, tag="dmx1")
                nc.vector.tensor_scalar_max(dmx[:], ops[:, fdim1:fdim1 + H1], 1e-30)
                r = ep.tile([128, H1], f32, tag="r1")
                nc.vector.reciprocal(r[:], dmx[:])
                ot = ep.tile([128, fdim1], f32, tag="ot")
                nc.vector.scalar_tensor_tensor(
                    out=ot[:], in0=ops[:, 0:fdim1], scalar=r[:, 0:1],
                    in1=res_sb[:, fdim1 * m:fdim1 * (m + 1)],
                    op0=mybir.AluOpType.mult, op1=mybir.AluOpType.add)
                nc.sync.dma_start(out_d[128 * m:128 * (m + 1), :], ot[:])

            edge_loop(t1_lo, t1_hi, ELEM1, fdim1, H1, er1_own, epi1)

    nc.compile()
    return nc



def kernel(x, src, dst, W0, al0, ar0, W1, al1, ar1, Wres):
    from concourse.bass_utils import run_bass_kernel_spmd

    x = np.asarray(x, np.float32)
    src = np.asarray(src, np.int32)
    dst = np.asarray(dst, np.int32)
    W0 = np.asarray(W0, np.float32)
    al0 = np.asarray(al0, np.float32)
    ar0 = np.asarray(ar0, np.float32)
    W1 = np.asarray(W1, np.float32)
    al1 = np.asarray(al1, np.float32)
    ar1 = np.asarray(ar1, np.float32)
    Wres = np.asarray(Wres, np.float32)

    N, F = x.shape
    NC = 8
    n_pad = ((N + NC * 128 - 1) // (NC * 128)) * (NC * 128)
    H0, D0 = al0.shape
    H1, D1 = al1.shape
    fdim0, fdim1 = H0 * D0, H1 * D1

    meta = _prep(src, dst, n_pad, NC, gh=GH)
    per = meta["per"]

    w0e = np.concatenate([
        W0,
        (W0.reshape(F, H0, D0) * al0[None]).sum(-1),
        (W0.reshape(F, H0, D0) * ar0[None]).sum(-1),
    ], axis=1).astype(np.float32)
    w1e = np.concatenate([
        W1,
        (W1.reshape(fdim0, H1, D1) * al1[None]).sum(-1),
        (W1.reshape(fdim0, H1, D1) * ar1[None]).sum(-1),
        Wres,
    ], axis=1).astype(np.float32)

    x_pad = np.zeros((n_pad, F), np.float32)
    x_pad[:N] = x
    xT = np.ascontiguousarray(x_pad.T)

    nc = _build(meta, fdim0, (H0, D0), fdim1, (H1, D1))

    in_maps = []
    for c in range(NC):
        ilo, dlo, dtlo = meta["cores"][c][0]
        ihi, dhi, dthi = meta["cores"][c][1]
        in_maps.append({
            "xts": np.ascontiguousarray(xT[:, c * per:(c + 1) * per]),
            "w0e": w0e, "w1e": w1e,
            "idxlo": ilo, "idxhi": ihi,
            "drlo": dlo, "drhi": dhi,
            "drtlo": dtlo, "drthi": dthi,
        })

    res = run_bass_kernel_spmd(nc, in_maps, core_ids=list(range(NC)))
    out = np.concatenate([res.results[c]["out"] for c in range(NC)], axis=0)
    kernel.last_exec_ns = res.exec_time_ns
    return out[:N].astype(np.float32)


kernel.last_exec_ns = None


# revision 11
# speedup vs baseline: 1.2423x; 1.0978x over previous
import sys
import numpy as np

sys.path.insert(0, "/opt/trn_rl_repo")

NEG = 0.2
ELEM0 = 384
ELEM1 = 128



def _prep(src, dst, n_pad, n_cores):
    import ml_dtypes

    per = n_pad // n_cores
    nmac = per // 128
    half_lo = ((nmac + 1) // 2) * 128
    half_hi = per - half_lo

    order = np.argsort(dst, kind="stable")
    ds = dst[order].astype(np.int64)
    ss = src[order].astype(np.int64)

    src_core = ss // per
    src_row = ss % per
    is_lo = src_row < half_lo
    rowid = np.where(is_lo,
                     half_lo * src_core + src_row,
                     half_hi * src_core + src_row - half_lo).astype(np.int64)
    mac = ds // 128
    drel = (ds - mac * 128).astype(np.int64)

    groups = {}
    counts = np.zeros((n_cores, nmac, 2), np.int64)
    for c in range(n_cores):
        base = c * nmac
        for half in (0, 1):
            sel_half = is_lo if half == 0 else ~is_lo
            for m in range(nmac):
                sel = (mac == base + m) & sel_half
                groups[(c, m, half)] = (rowid[sel], drel[sel])
                counts[c, m, half] = sel.sum()

    G = np.ceil(counts.max(axis=0) / 128).astype(np.int64)
    G[:, 0] = np.where(G.sum(axis=1) == 0, 1, G[:, 0])

    subs = {0: [], 1: []}
    for m in range(nmac):
        for half in (0, 1):
            for j in range(G[m, half]):
                subs[half].append((m, j))
    nsub = {h: len(subs[h]) for h in (0, 1)}
    ncall = {h: (nsub[h] + 7) // 8 for h in (0, 1)}

    mac_subs = {m: [] for m in range(nmac)}
    for h in (0, 1):
        for p, (m, j) in enumerate(subs[h]):
            seq = 2 * (p // 8) + h
            mac_subs[m].append((seq, p % 8, h, p))
    flags = {}
    close_after = {}
    for m, lst in mac_subs.items():
        lst.sort()
        fk = (lst[0][2], lst[0][3])
        lk = (lst[-1][2], lst[-1][3])
        flags.setdefault(fk, [False, False])[0] = True
        flags.setdefault(lk, [False, False])[1] = True
        close_after.setdefault(lst[-1][0], []).append(m)

    cores = {}
    for c in range(n_cores):
        out = {}
        for half in (0, 1):
            ns_, nc_ = nsub[half], ncall[half]
            iv = np.zeros(ns_ * 128, np.int64)
            dv = np.full(ns_ * 128, -1.0, np.float32)
            for p, (m, j) in enumerate(subs[half]):
                rid, dr = groups[(c, m, half)]
                a = j * 128
                seg_r = rid[a:a + 128]
                seg_d = dr[a:a + 128]
                iv[p * 128:p * 128 + len(seg_r)] = seg_r
                dv[p * 128:p * 128 + len(seg_d)] = seg_d
            idx_arr = np.zeros((128, nc_ * 64), np.int16)
            dr_arr = np.full((128, nc_ * 8), -1.0, np.float32)
            drt_arr = np.full((128, nc_ * 1024), -1.0, np.float32)
            for k in range(nc_):
                t0 = k * 1024
                nt = min(1024, ns_ * 128 - t0)
                buf = np.zeros(1024, np.int64)
                buf[:nt] = iv[t0:t0 + nt]
                idx_arr[:, k * 64:(k + 1) * 64] = np.tile(
                    buf.reshape(64, 16).T.astype(np.int16), (8, 1))
                dbuf = np.full(1024, -1.0, np.float32)
                dbuf[:nt] = dv[t0:t0 + nt]
                dr_arr[:, k * 8:(k + 1) * 8] = dbuf.reshape(8, 128).T
                drt_arr[:, k * 1024:(k + 1) * 1024] = dbuf[None, :]
            out[half] = (idx_arr, dr_arr.astype(ml_dtypes.bfloat16),
                         drt_arr.astype(ml_dtypes.bfloat16))
        cores[c] = out

    return {
        "per": per, "half_lo": half_lo, "half_hi": half_hi, "nmac": nmac,
        "subs": subs, "nsub": nsub, "ncall": ncall,
        "flags": flags, "close_after": close_after,
        "cores": cores,
        "tokens": (nsub[0] + nsub[1]) * 128,
    }



def _build(meta, fdim0, hd0, fdim1, hd1):
    import concourse.bacc as bacc
    import concourse.mybir as mybir
    import concourse.tile as tile
    from concourse.masks import make_identity

    NC = 8
    per = meta["per"]
    half_lo, half_hi = meta["half_lo"], meta["half_hi"]
    nmac = meta["nmac"]
    subs, nsub, ncall = meta["subs"], meta["nsub"], meta["ncall"]
    flags, close_after = meta["flags"], meta["close_after"]

    H0, D0 = hd0
    H1, D1 = hd1
    C0 = fdim0 + H0
    C1 = fdim1 + H1
    W0N = fdim0 + 2 * H0
    W1N = fdim1 + 2 * H1 + fdim1
    KD = fdim0
    NK = KD // 128

    f32 = mybir.dt.float32
    bf16 = mybir.dt.bfloat16
    i16 = mybir.dt.int16

    nc = bacc.Bacc("TRN2", target_bir_lowering=False, debug=False,
                   num_devices=NC, num_swdge_queues=4,
                   dynamic_dma_scratch_size=65536)

    xts = nc.declare_dram_parameter("xts", [KD, per], f32, isOutput=False)
    w0e = nc.declare_dram_parameter("w0e", [KD, W0N], f32, isOutput=False)
    w1e = nc.declare_dram_parameter("w1e", [fdim0, W1N], f32, isOutput=False)
    idx_d, dr_d, drt_d = {}, {}, {}
    for h, nm in ((0, "lo"), (1, "hi")):
        idx_d[h] = nc.declare_dram_parameter(f"idx{nm}", [128, ncall[h] * 64],
                                             i16, isOutput=False)
        dr_d[h] = nc.declare_dram_parameter(f"dr{nm}", [128, ncall[h] * 8],
                                            bf16, isOutput=False)
        drt_d[h] = nc.declare_dram_parameter(f"drt{nm}", [128, ncall[h] * 1024],
                                             bf16, isOutput=False)
    out_d = nc.declare_dram_parameter("out", [per, fdim1], f32, isOutput=True)

    t0_own = nc.dram_tensor("t0_own", [per, ELEM0], bf16)
    t0_half = [nc.dram_tensor("t0_lo", [NC * half_lo, ELEM0], bf16,
                              addr_space="Shared"),
               nc.dram_tensor("t0_hi", [NC * half_hi, ELEM0], bf16,
                              addr_space="Shared")]
    t1_own = nc.dram_tensor("t1_own", [per, ELEM1], bf16)
    t1_half = [nc.dram_tensor("t1_lo", [NC * half_lo, ELEM1], bf16,
                              addr_space="Shared"),
               nc.dram_tensor("t1_hi", [NC * half_hi, ELEM1], bf16,
                              addr_space="Shared")]
    er0_own = nc.dram_tensor("er0_own", [per, H0], f32)
    er1_own = nc.dram_tensor("er1_own", [per, H1], f32)

    rg = [list(range(NC))]

    def call_slots(h, k):
        out = []
        for s in range(8):
            p = 8 * k + s
            if p >= nsub[h]:
                break
            m, _ = subs[h][p]
            fl = flags.get((h, p), [False, False])
            out.append((s, m, fl[0], fl[1]))
        return out

    with tile.TileContext(nc) as tc:
        with tc.tile_pool(name="consts", bufs=1) as consts:
            ident_bf = consts.tile([128, 128], bf16)
            make_identity(nc, ident_bf[:])
            iota_bf = consts.tile([128, 128], bf16)
            nc.gpsimd.iota(iota_bf[:], pattern=[[1, 128]], base=0,
                           channel_multiplier=0,
                           allow_small_or_imprecise_dtypes=True)
            iota_col = consts.tile([128, 1], f32)
            nc.gpsimd.iota(iota_col[:], pattern=[[1, 1]], base=0,
                           channel_multiplier=1,
                           allow_small_or_imprecise_dtypes=True)
            w0s = consts.tile([128, NK * W0N], f32)
            for j in range(NK):
                nc.sync.dma_start(w0s[:, j * W0N:(j + 1) * W0N],
                                  w0e[128 * j:128 * (j + 1), :])
            w1s = consts.tile([128, NK * W1N], f32)
            for j in range(NK):
                nc.sync.dma_start(w1s[:, j * W1N:(j + 1) * W1N],
                                  w1e[128 * j:128 * (j + 1), :])
            w1sb = consts.tile([128, NK * W1N], bf16)
            nc.vector.tensor_copy(w1sb[:], w1s[:])
            idx_t, dr_t = {}, {}
            for h in (0, 1):
                idx_t[h] = consts.tile([128, ncall[h] * 64], i16, name=f"idx{h}")
                nc.sync.dma_start(idx_t[h][:], idx_d[h][:])
                dr_t[h] = consts.tile([128, ncall[h] * 8], bf16, name=f"dr{h}")
                nc.sync.dma_start(dr_t[h][:], dr_d[h][:])
            res_sb = consts.tile([128, nmac * fdim1], f32)

            with (
                tc.tile_pool(name="pa", bufs=3) as pa,
                tc.tile_pool(name="paps", bufs=2, space="PSUM") as paps,
            ):
                for t in range(nmac):
                    ps = paps.tile([128, W0N], f32, tag="psA")
                    for j in range(NK):
                        xt = pa.tile([128, 128], f32, tag="xt")
                        nc.sync.dma_start(
                            xt[:],
                            xts[128 * j:128 * (j + 1), 128 * t:128 * (t + 1)])
                        nc.tensor.matmul(ps[:], lhsT=xt[:],
                                         rhs=w0s[:, j * W0N:(j + 1) * W0N],
                                         start=(j == 0), stop=(j == NK - 1))
                    sb = pa.tile([128, ELEM0], bf16, tag="sbA")
                    nc.scalar.memzero(sb[:, C0:ELEM0])
                    nc.vector.tensor_copy(sb[:, 0:C0], ps[:, 0:C0])
                    nc.sync.dma_start(t0_own[128 * t:128 * (t + 1), :], sb[:])
                    se = pa.tile([128, H0], f32, tag="seA")
                    nc.vector.tensor_copy(se[:], ps[:, fdim0 + H0:fdim0 + 2 * H0])
                    nc.sync.dma_start(er0_own[128 * t:128 * (t + 1), :], se[:])
                    if 128 * (t + 1) == half_lo:
                        nc.gpsimd.collective_compute(
                            "AllGather", mybir.AluOpType.bypass,
                            replica_groups=rg,
                            ins=[t0_own[0:half_lo, :]], outs=[t0_half[0][:]])
                nc.gpsimd.collective_compute(
                    "AllGather", mybir.AluOpType.bypass, replica_groups=rg,
                    ins=[t0_own[half_lo:per, :]], outs=[t0_half[1][:]])

            qctr = [0]

            def edge_loop(t_half, elem, fdim, H, er_own, epilogue):
                D = fdim // H
                C = fdim + H
                with (
                    tc.tile_pool(name="gp", bufs=6) as gp,
                    tc.tile_pool(name="wp", bufs=3) as wp,
                    tc.tile_pool(name="ep", bufs=3) as ep,
                    tc.tile_pool(name="pps", bufs=2, space="PSUM") as pps,
                    tc.tile_pool(name="opsp", bufs=2, space="PSUM") as opsp,
                ):
                    er_bands = {}
                    out_ps = {}

                    def ensure_macro(m):
                        if m in out_ps:
                            return
                        out_ps[m] = opsp.tile([128, C], f32, tag="outps",
                                              name=f"ops{m}")
                        eb = ep.tile([128, H], f32, tag="erb")
                        nc.sync.dma_start(eb[:], er_own[128 * m:128 * (m + 1), :])
                        ebb = ep.tile([128, H], bf16, tag="erbb")
                        nc.vector.tensor_copy(ebb[:], eb[:])
                        er_bands[m] = ebb

                    def emit_call(h, k):
                        slots = call_slots(h, k)
                        if not slots:
                            return
                        ns = len(slots)
                        nt = 128 * ns
                        for (_, m, _, _) in slots:
                            ensure_macro(m)
                        g = gp.tile([128, 8 * elem], bf16, tag="g")
                        nc.gpsimd.dma_gather(
                            idxs_ap=idx_t[h][:, 64 * k:64 * k + (nt // 16)],
                            in_ap=t_half[h][:],
                            out_ap=g[:, 0:ns * elem].rearrange(
                                "p (s e) -> p s e", e=elem),
                            num_idxs=nt, num_idxs_reg=nt, elem_size=elem,
                            queue_num=qctr[0] % 4,
                        )
                        qctr[0] += 1
                        gv = g[:].rearrange("p (s e) -> p s e", e=elem)

                        B = wp.tile([128, 8 * 128], bf16, tag="B")
                        nc.vector.tensor_tensor(
                            out=B[:, 0:ns * 128].rearrange(
                                "p (s n) -> p s n", n=128),
                            in0=iota_bf[:].unsqueeze(1)
                                .broadcast_to([128, ns, 128]),
                            in1=dr_t[h][:, 8 * k:8 * k + ns].unsqueeze(2)
                                .broadcast_to([128, ns, 128]),
                            op=mybir.AluOpType.is_equal)
                        drt = wp.tile([128, 8 * 128], bf16, tag="drt")
                        nc.sync.dma_start(
                            drt[:, 0:ns * 128],
                            drt_d[h][:, 1024 * k:1024 * k + ns * 128])
                        BT = wp.tile([128, 8 * 128], bf16, tag="BT")
                        nc.vector.tensor_scalar(
                            out=BT[:, 0:ns * 128], in0=drt[:, 0:ns * 128],
                            scalar1=iota_col[:, 0:1], scalar2=None,
                            op0=mybir.AluOpType.is_equal)

                        erp = pps.tile([128, 8 * H], f32, tag="small")
                        for (s, m, _, _) in slots:
                            nc.tensor.matmul(
                                erp[:, s * H:(s + 1) * H],
                                lhsT=BT[:, s * 128:(s + 1) * 128],
                                rhs=er_bands[m][:], start=True, stop=True)

                        e_t = ep.tile([128, 8 * H], f32, tag="e")
                        nc.vector.tensor_tensor(
                            out=e_t[:, 0:ns * H].rearrange(
                                "p (s h) -> p s h", h=H),
                            in0=gv[:, 0:ns, fdim:fdim + H],
                            in1=erp[:, 0:ns * H].rearrange(
                                "p (s h) -> p s h", h=H),
                            op=mybir.AluOpType.add)
                        ab = ep.tile([128, 8 * H], f32, tag="abs")
                        nc.scalar.activation(ab[:, 0:ns * H], e_t[:, 0:ns * H],
                                             mybir.ActivationFunctionType.Abs,
                                             scale=(1.0 - NEG) / 2.0)
                        lr = ep.tile([128, 8 * H], f32, tag="lr")
                        nc.vector.scalar_tensor_tensor(
                            out=lr[:, 0:ns * H], in0=e_t[:, 0:ns * H],
                            scalar=(1.0 + NEG) / 2.0, in1=ab[:, 0:ns * H],
                            op0=mybir.AluOpType.mult, op1=mybir.AluOpType.add)

                        rhs = wp.tile([128, 8 * C], bf16, tag="rhs")
                        rv = rhs[:].rearrange("p (s c) -> p s c", c=C)
                        nc.scalar.activation(rv[:, 0:ns, fdim:C],
                                             lr[:, 0:ns * H].rearrange(
                                                 "p (s h) -> p s h", h=H),
                                             mybir.ActivationFunctionType.Exp)
                        nc.vector.tensor_tensor(
                            out=rv[:, 0:ns, 0:fdim].rearrange(
                                "p s (h d) -> p s h d", d=D),
                            in0=gv[:, 0:ns, 0:fdim].rearrange(
                                "p s (h d) -> p s h d", d=D),
                            in1=rv[:, 0:ns, fdim:C]
                                .unsqueeze(3).broadcast_to([128, ns, H, D]),
                            op=mybir.AluOpType.mult)

                        for (s, m, first, last) in slots:
                            nc.tensor.matmul(
                                out_ps[m][:], lhsT=B[:, s * 128:(s + 1) * 128],
                                rhs=rhs[:, s * C:(s + 1) * C],
                                start=first, stop=last)

                    kmax = max(ncall[0], ncall[1])
                    for k in range(kmax):
                        for h in (0, 1):
                            if k < ncall[h]:
                                emit_call(h, k)
                                for m in close_after.get(2 * k + h, []):
                                    epilogue(m, out_ps.pop(m), pps, ep)

            closed0 = set()

            def epi0(m, ops, pps, ep):
                dmx = ep.tile([128, H0], f32, tag="dmx")
                nc.vector.tensor_scalar_max(dmx[:], ops[:, fdim0:fdim0 + H0],
                                            1e-30)
                r = ep.tile([128, H0], f32, tag="r")
                nc.vector.reciprocal(r[:], dmx[:])
                hn = ep.tile([128, fdim0], f32, tag="hn")
                nc.vector.tensor_tensor(
                    out=hn[:].rearrange("p (h d) -> p h d", d=D0),
                    in0=ops[:, 0:fdim0].rearrange("p (h d) -> p h d", d=D0),
                    in1=r[:].unsqueeze(2).broadcast_to([128, H0, D0]),
                    op=mybir.AluOpType.mult)
                mt = ep.tile([128, fdim0], f32, tag="mt")
                nc.scalar.activation(mt[:], hn[:],
                                     mybir.ActivationFunctionType.Relu,
                                     scale=-1.0)
                em = ep.tile([128, fdim0], f32, tag="em")
                nc.scalar.activation(em[:], mt[:],
                                     mybir.ActivationFunctionType.Exp,
                                     scale=-1.0)
                pt = ep.tile([128, fdim0], f32, tag="pt")
                nc.scalar.activation(pt[:], hn[:],
                                     mybir.ActivationFunctionType.Relu)
                h_ = ep.tile([128, fdim0], f32, tag="h_")
                nc.vector.scalar_tensor_tensor(
                    out=h_[:], in0=em[:], scalar=-1.0, in1=pt[:],
                    op0=mybir.AluOpType.add, op1=mybir.AluOpType.add)

                h_bf = ep.tile([128, fdim0], bf16, tag="h_bf")
                nc.vector.tensor_copy(h_bf[:], h_[:])
                ht_ps = pps.tile([128, 512], bf16, tag="btps")
                for j in range(NK):
                    nc.tensor.transpose(out=ht_ps[:, j * 128:(j + 1) * 128],
                                        in_=h_bf[:, j * 128:(j + 1) * 128],
                                        identity=ident_bf[:])
                ht = ep.tile([128, fdim0], bf16, tag="ht")
                nc.vector.tensor_copy(ht[:], ht_ps[:, 0:fdim0])
                ps1 = pps.tile([128, W1N], f32, tag="small2")
                for j in range(NK):
                    nc.tensor.matmul(ps1[:], lhsT=ht[:, j * 128:(j + 1) * 128],
                                     rhs=w1sb[:, j * W1N:(j + 1) * W1N],
                                     start=(j == 0), stop=(j == NK - 1))
                t1row = ep.tile([128, ELEM1], bf16, tag="t1row")
                nc.vector.memset(t1row[:, C1:ELEM1], 0.0)
                nc.vector.tensor_copy(t1row[:, 0:C1], ps1[:, 0:C1])
                nc.sync.dma_start(t1_own[128 * m:128 * (m + 1), :], t1row[:])
                se1 = ep.tile([128, H1], f32, tag="se1")
                nc.vector.tensor_copy(se1[:], ps1[:, fdim1 + H1:fdim1 + 2 * H1])
                nc.sync.dma_start(er1_own[128 * m:128 * (m + 1), :], se1[:])
                nc.vector.tensor_copy(
                    res_sb[:, fdim1 * m:fdim1 * (m + 1)],
                    ps1[:, fdim1 + 2 * H1:W1N])
                closed0.add(m)
                if "aglo" not in closed0 and \
                        all(mm in closed0 for mm in range(half_lo // 128)):
                    closed0.add("aglo")
                    nc.gpsimd.collective_compute(
                        "AllGather", mybir.AluOpType.bypass, replica_groups=rg,
                        ins=[t1_own[0:half_lo, :]], outs=[t1_half[0][:]])

            edge_loop(t0_half, ELEM0, fdim0, H0, er0_own, epi0)

            nc.gpsimd.collective_compute(
                "AllGather", mybir.AluOpType.bypass, replica_groups=rg,
                ins=[t1_own[half_lo:per, :]], outs=[t1_half[1][:]])

            def epi1(m, ops, pps, ep):
                dmx = ep.tile([128, H1], f32, tag="dmx1")
                nc.vector.tensor_scalar_max(dmx[:], ops[:, fdim1:fdim1 + H1],
                                            1e-30)
                r = ep.tile([128, H1], f32, tag="r1")
                nc.vector.reciprocal(r[:], dmx[:])
                ot = ep.tile([128, fdim1], f32, tag="ot")
                nc.vector.scalar_tensor_tensor(
                    out=ot[:], in0=ops[:, 0:fdim1], scalar=r[:, 0:1],
                    in1=res_sb[:, fdim1 * m:fdim1 * (m + 1)],
                    op0=mybir.AluOpType.mult, op1=mybir.AluOpType.add)
                nc.sync.dma_start(out_d[128 * m:128 * (m + 1), :], ot[:])

            edge_loop(t1_half, ELEM1, fdim1, H1, er1_own, epi1)

    nc.compile()
    return nc



def kernel(x, src, dst, W0, al0, ar0, W1, al1, ar1, Wres):
    from concourse.bass_utils import run_bass_kernel_spmd

    x = np.asarray(x, np.float32)
    src = np.asarray(src, np.int32)
    dst = np.asarray(dst, np.int32)
    W0 = np.asarray(W0, np.float32)
    al0 = np.asarray(al0, np.float32)
    ar0 = np.asarray(ar0, np.float32)
    W1 = np.asarray(W1, np.float32)
    al1 = np.asarray(al1, np.float32)
    ar1 = np.asarray(ar1, np.float32)
    Wres = np.asarray(Wres, np.float32)

    N, F = x.shape
    NC = 8
    n_pad = ((N + NC * 128 - 1) // (NC * 128)) * (NC * 128)
    H0, D0 = al0.shape
    H1, D1 = al1.shape
    fdim0, fdim1 = H0 * D0, H1 * D1

    meta = _prep(src, dst, n_pad, NC)
    per = meta["per"]

    w0e = np.concatenate([
        W0,
        (W0.reshape(F, H0, D0) * al0[None]).sum(-1),
        (W0.reshape(F, H0, D0) * ar0[None]).sum(-1),
    ], axis=1).astype(np.float32)
    w1e = np.concatenate([
        W1,
        (W1.reshape(fdim0, H1, D1) * al1[None]).sum(-1),
        (W1.reshape(fdim0, H1, D1) * ar1[None]).sum(-1),
        Wres,
    ], axis=1).astype(np.float32)

    x_pad = np.zeros((n_pad, F), np.float32)
    x_pad[:N] = x
    xT = np.ascontiguousarray(x_pad.T)

    nc = _build(meta, fdim0, (H0, D0), fdim1, (H1, D1))

    in_maps = []
    for c in range(NC):
        ilo, dlo, dtlo = meta["cores"][c][0]
        ihi, dhi, dthi = meta["cores"][c][1]
        in_maps.append({
            "xts": np.ascontiguousarray(xT[:, c * per:(c + 1) * per]),
            "w0e": w0e, "w1e": w1e,
            "idxlo": ilo, "idxhi": ihi,
            "drlo": dlo, "drhi": dhi,
            "drtlo": dtlo, "drthi": dthi,
        })

    res = run_bass_kernel_spmd(nc, in_maps, core_ids=list(range(NC)))
    out = np.concatenate([res.results[c]["out"] for c in range(NC)], axis=0)
    kernel.last_exec_ns = res.exec_time_ns
    return out[:N].astype(np.float32)


kernel.last_exec_ns = None
